# revision 8
# baseline (speedup 1.0000x reference)
"""Causal self-attention (B=4, T=2048, C=1024, 16 heads x 64) on 8 TRN2 NeuronCores.

Sharding: tensor-parallel over heads. Core c owns heads {2c, 2c+1}:
  - w_attn column slices -> per-core QKV in transposed layout (dims on
    partitions, tokens on free dim),
  - attention in S^T form: S^T[k,q] = matmul(lhsT=kT, rhs=qT_headzeroed),
    softmax denominator via ones-columns appended to V, PV consumes exp(S^T)
    directly, partial output projection in transposed layout,
  - host sums the 8 partial projections (the TP all-reduce).

v3 (fused pipeline, engine-balanced):
  - single fused QKV+attention phase: QKV chunks of batch b+1 are
    interleaved between attention chunks of batch b, giving the PE extra
    work that covers ScalarE Exp latency (ScalarE is a hard 1 elem/cycle
    @1.2GHz with no accel modes -> ~157us of Exp; PE alone in the
    attention loop cannot outrun it, PE+QKV can),
  - ScalarE runs ONLY Exp (one merged activation per k-block: q tiles for
    both heads live in one [128,1024] psum tile),
  - causal masking via gpsimd affine_select on the exp'd probabilities
    (GpSimd is otherwise idle; DVE mask-muls removed),
  - PSUM->SBUF moves (psY->sbY) via DMA instead of DVE,
  - all PSUM evictions on DVE; deferred-by-1 normalize broadcast and
    deferred-by-2 projection keep every PE instruction's deps ready
    before the in-order PE queue reaches it,
  - bf16 operands everywhere (PSUM accum f32), f32r for the tiny
    normalize path.
"""

import sys
import numpy as np

sys.path.insert(0, "/opt/trn_rl_repo")

B, T, C = 4, 2048, 1024
NH, HD = 16, 64
NCORES = 8
TOK = B * T                 # 8192 tokens
NCH = TOK // 512            # 16 token chunks of 512
CHB = T // 512              # 4 chunks per batch
NKB_B = T // 128            # 16 k-blocks per batch
SCALE = 1.0 / 8.0

_CACHE = {}


def _build_program():
    import concourse.tile as tile
    from concourse import bacc, mybir
    from concourse.masks import make_identity

    f32 = mybir.dt.float32
    f32r = mybir.dt.float32r
    bf16 = mybir.dt.bfloat16

    nc = bacc.Bacc("TRN2", target_bir_lowering=False, debug=False,
                   num_devices=NCORES)

    xT = nc.dram_tensor("xT", [C, TOK], bf16, kind="ExternalInput").ap()
    wqkv = nc.dram_tensor("wqkv", [C, 384], bf16, kind="ExternalInput").ap()
    battn = nc.dram_tensor("battn", [128, 3], f32, kind="ExternalInput").ap()
    wproj = nc.dram_tensor("wproj", [128, C], bf16, kind="ExternalInput").ap()
    bproj = nc.dram_tensor("bproj", [128, 8], f32, kind="ExternalInput").ap()
    outT = nc.dram_tensor("outT", [C, TOK], bf16, kind="ExternalOutput").ap()

    with tile.TileContext(nc) as tc:
        with tc.tile_pool(name="const", bufs=1) as const, \
             tc.tile_pool(name="resid", bufs=1) as resid:
            w_sb = const.tile([128, 8, 384], bf16, tag="w_sb")
            nc.sync.dma_start(w_sb[:], wqkv.rearrange("(ko p) m -> p ko m", p=128))

            ident = const.tile([128, 128], f32, tag="ident")
            make_identity(nc, ident)
            bias = const.tile([128, 11], f32, tag="bias")
            battn_sb = bias[:, 0:3]
            bp_sb = bias[:, 3:11]
            nc.sync.dma_start(battn_sb, battn[:])
            nc.sync.dma_start(bp_sb, bproj[:])
            wp_sb = const.tile([128, C], bf16, tag="wp")
            nc.sync.dma_start(wp_sb[:], wproj[:])
            ones_r = const.tile([1, 128], f32r, tag="ones")
            nc.gpsimd.memset(ones_r.bitcast(f32), 1.0)

            # resident activations
            kT = resid.tile([128, NCH, 512], bf16, tag="kT")
            # qz: both heads' q in one tile; [:, :, 0:512] = head0 slots
            # (partitions 64:128 zero), [:, :, 512:1024] = head1 slots
            # (partitions 0:64 zero) -> one 1024-row S matmul per k-block
            qz = resid.tile([128, NCH, 1024], bf16, tag="qz")
            vpr = resid.tile([128, 64, 132], bf16, tag="vpr")
            vprv = vpr.rearrange("p kb (h c) -> p kb h c", c=66)

            nc.vector.memset(qz[64:128, :, 0:512], 0.0)
            nc.gpsimd.memset(qz[0:64, :, 512:1024], 0.0)
            nc.gpsimd.memset(vprv[:, :, :, 64:66], 1.0)

            xTr = xT.rearrange("(ko p) t -> p ko t", p=128)
            outTr = outT.rearrange("(od p) t -> p od t", p=128)

            with tc.tile_pool(name="xin", bufs=4) as xin, \
                 tc.tile_pool(name="vsp", bufs=2) as vsp, \
                 tc.tile_pool(name="pss", bufs=3, space="PSUM") as pssp, \
                 tc.tile_pool(name="psy", bufs=1, space="PSUM") as psyp, \
                 tc.tile_pool(name="pb", bufs=3) as pbp, \
                 tc.tile_pool(name="sby", bufs=2) as sbyp, \
                 tc.tile_pool(name="nrm", bufs=2) as nrm, \
                 tc.tile_pool(name="ytc", bufs=3) as ytc, \
                 tc.tile_pool(name="ob", bufs=2) as obp:

                def emit_A_mm(ch):
                    """QKV matmuls + q/k/v evictions for one 512-token chunk.
                    V transposes are deferred (emit_A_transp)."""
                    xa = xin.tile([128, 4, 512], bf16, tag="xc")
                    nc.sync.dma_start(xa[:], xTr[:, 0:4, ch * 512:(ch + 1) * 512])
                    xb = xin.tile([128, 4, 512], bf16, tag="xc")
                    nc.sync.dma_start(xb[:], xTr[:, 4:8, ch * 512:(ch + 1) * 512])
                    tQK = pssp.tile([128, 1024], f32, tag="s")
                    for half, m in ((0, 0), (1, 1)):
                        for ko in range(8):
                            xsrc = xa if ko < 4 else xb
                            nc.tensor.matmul(tQK[:, half * 512:(half + 1) * 512],
                                             w_sb[:, ko, m * 128:(m + 1) * 128],
                                             xsrc[:, ko % 4, :],
                                             start=(ko == 0), stop=(ko == 7))
                    nc.vector.tensor_scalar_add(qz[0:64, ch, 0:512],
                                                tQK[0:64, 0:512], battn_sb[0:64, 0:1])
                    nc.vector.tensor_scalar_add(qz[64:128, ch, 512:1024],
                                                tQK[64:128, 0:512], battn_sb[64:128, 0:1])
                    nc.vector.tensor_scalar_add(kT[:, ch, :], tQK[:, 512:1024],
                                                battn_sb[:, 1:2])
                    tV = pssp.tile([128, 1024], f32, tag="s")
                    for ko in range(8):
                        xsrc = xa if ko < 4 else xb
                        nc.tensor.matmul(tV[:, 0:512],
                                         w_sb[:, ko, 256:384],
                                         xsrc[:, ko % 4, :],
                                         start=(ko == 0), stop=(ko == 7))
                    vs = vsp.tile([128, 512], f32, tag="vs")
                    nc.vector.tensor_scalar_add(vs[:], tV[:, 0:512], battn_sb[:, 2:3])
                    return ch, vs, tV

                def emit_A_transp(ch, vs, tV):
                    # PE transposes into the back half of tV (vs eviction has
                    # long completed by the time the PE reaches these)
                    tVt = tV.rearrange("p (tb d) -> p tb d", d=128)
                    for t in range(4):
                        nc.tensor.transpose(tVt[:, 4 + t, :],
                                            vs[:, t * 128:(t + 1) * 128], ident)
                        nc.vector.tensor_copy(
                            vprv[:, ch * 4 + t, :, 0:64],
                            tVt[:, 4 + t, :].rearrange("p (h d) -> p h d", d=64))

                def emit_S(b, j, qch, kb):
                    vstart = max(0, kb * 128 - j * 512)
                    kch = b * CHB + kb // 4
                    ksub = (kb % 4) * 128
                    # matmul moving dim is capped at 512 (one PSUM bank), so
                    # one matmul per head into halves of a shared tile; the
                    # merged Exp then covers both halves in one activation
                    s = pssp.tile([128, 1024], f32, tag="s")
                    for h in range(2):
                        nc.tensor.matmul(
                            s[:, h * 512 + vstart:(h + 1) * 512],
                            kT[:, kch, ksub:ksub + 128],
                            qz[:, qch, h * 512 + vstart:(h + 1) * 512],
                            start=True, stop=True)
                    return s, vstart

                def emit_exp(j, kb, s, vstart):
                    p = pbp.tile([128, 1024], bf16, tag="p")
                    pv = p.rearrange("p (h q) -> p h q", q=512)
                    sv = s.rearrange("p (h q) -> p h q", q=512)
                    nc.scalar.activation(pv[:, :, vstart:], sv[:, :, vstart:],
                                         mybir.ActivationFunctionType.Exp, scale=SCALE)
                    if kb >= 4 * j:
                        # inline causal mask on GpSimd: keep where k <= q
                        nc.gpsimd.affine_select(
                            out=pv[:, :, vstart:vstart + 128],
                            in_=pv[:, :, vstart:vstart + 128],
                            compare_op=mybir.AluOpType.is_gt,
                            fill=0.0, base=1,
                            pattern=[[0, 2], [1, 128]], channel_multiplier=-1,
                        )
                    return p, vstart

                def emit_epilogue(qch, psY):
                    sbY = sbyp.tile([66, 1024], f32, tag="sby")
                    nc.vector.tensor_copy(sbY[:], psY[0:66, :, :])
                    sc = nrm.tile([128, 16], f32, tag="sc")
                    nc.sync.dma_start(sc[:, 0:8], sbY[64:65, :])
                    nc.vector.reciprocal(sc[:, 8:16], sc[:, 0:8])
                    rr = nrm.tile([1, 1024], f32r, tag="rr")
                    nc.sync.dma_start(rr[0:1, :], sc[:, 8:16].bitcast(f32r))
                    return sbY, rr

                def emit_norm(qch, sbY, rr):
                    r = pssp.tile([128, 1024], f32, tag="s")
                    nc.tensor.matmul(r[:, 0:512], ones_r[0:1, :], rr[0:1, 0:512],
                                     start=True, stop=True)
                    nc.tensor.matmul(r[:, 512:1024], ones_r[0:1, :], rr[0:1, 512:1024],
                                     start=True, stop=True)
                    yTch = ytc.tile([128, 512], bf16, tag="yt")
                    nc.vector.tensor_mul(yTch[0:64, :], sbY[0:64, 0:512], r[0:64, 0:512])
                    yst = nrm.tile([64, 512], bf16, tag="yst")
                    nc.vector.tensor_mul(yst[:], sbY[0:64, 512:1024], r[0:64, 512:1024])
                    nc.sync.dma_start(yTch[64:128, :], yst[:])
                    return yTch

                def emit_proj(qch, yTch):
                    oSb = obp.tile([128, 8, 512], bf16, tag="o")
                    for odp in range(4):
                        tP = pssp.tile([128, 1024], f32, tag="s")
                        for h in range(2):
                            od = odp * 2 + h
                            nc.tensor.matmul(tP[:, h * 512:(h + 1) * 512],
                                             wp_sb[:, od * 128:(od + 1) * 128],
                                             yTch[:], start=True, stop=True)
                            nc.vector.tensor_scalar_add(oSb[:, od, :],
                                                        tP[:, h * 512:(h + 1) * 512],
                                                        bp_sb[:, od:od + 1])
                    nc.sync.dma_start(outTr[:, :, qch * 512:(qch + 1) * 512], oSb[:])

                # ---------------- fused schedule ----------------
                # QKV for batch 0 up front; then per attention chunk of
                # batch b, interleave one QKV chunk of batch b+1.
                pend_tr = []     # [(ch, vs, tV)] QKV chunks awaiting transposes
                pend_norm = []   # [(qch, sbY, rr)]
                pend_proj = []   # [(qch, yTch)]
                for ch in range(CHB):
                    ep = emit_A_mm(ch)
                    if pend_tr:
                        emit_A_transp(*pend_tr.pop(0))
                    pend_tr.append(ep)

                for b in range(B):
                    for j in range(CHB):
                        qch = b * CHB + j
                        psY = psyp.tile([128, 2, 512], f32, tag="y")
                        nkb = 4 * j + 4

                        sq = [emit_exp(j, 0, *emit_S(b, j, qch, 0)),
                              emit_exp(j, 1, *emit_S(b, j, qch, 1))]
                        for kb in range(nkb):
                            p, vstart = sq.pop(0)
                            pv = p.rearrange("p (h q) -> p h q", q=512)
                            gkb = b * NKB_B + kb
                            for h in range(2):
                                nc.tensor.matmul(psY[0:66, h, vstart:],
                                                 vprv[:, gkb, h, :],
                                                 pv[:, h, vstart:],
                                                 start=(kb == 0), stop=(kb == nkb - 1))
                            if kb + 2 < nkb:
                                sq.append(emit_exp(j, kb + 2,
                                                   *emit_S(b, j, qch, kb + 2)))

                        ep = emit_epilogue(qch, psY)
                        # QKV chunk of the next batch: PE work whose deps are
                        # ready; also covers this chunk's epilogue DMA chain
                        if b + 1 < B:
                            at = emit_A_mm(b * CHB + CHB + j)
                            if pend_tr:
                                emit_A_transp(*pend_tr.pop(0))
                            pend_tr.append(at)
                        elif pend_tr:
                            emit_A_transp(*pend_tr.pop(0))
                        # deferred PE work (deps ready since last iteration)
                        if len(pend_proj) >= 2:
                            emit_proj(*pend_proj.pop(0))
                        if pend_norm:
                            pq, psbY, prr = pend_norm.pop(0)
                            pend_proj.append((pq, emit_norm(pq, psbY, prr)))
                        pend_norm.append((qch, *ep))
                # drain
                while pend_tr:
                    emit_A_transp(*pend_tr.pop(0))
                while pend_proj:
                    emit_proj(*pend_proj.pop(0))
                for pq, psbY, prr in pend_norm:
                    emit_proj(pq, emit_norm(pq, psbY, prr))

    nc.compile()
    return nc


def _get_program():
    if "nc" not in _CACHE:
        _CACHE["nc"] = _build_program()
    return _CACHE["nc"]


def kernel(x, w_attn, b_attn, w_proj, b_proj, _trace=False):
    import ml_dtypes
    from concourse.bass_utils import run_bass_kernel_spmd

    bf16 = ml_dtypes.bfloat16
    nc = _get_program()

    x = np.asarray(x, dtype=np.float32)
    w_attn = np.asarray(w_attn, dtype=np.float32)
    b_attn = np.asarray(b_attn, dtype=np.float32)
    w_proj = np.asarray(w_proj, dtype=np.float32)
    b_proj = np.asarray(b_proj, dtype=np.float32)

    xT_np = np.ascontiguousarray(x.reshape(TOK, C).T.astype(bf16))

    in_maps = []
    for c in range(NCORES):
        lo, hi = c * 128, (c + 1) * 128
        wq = w_attn[:, lo:hi]
        wk = w_attn[:, C + lo:C + hi]
        wv = w_attn[:, 2 * C + lo:2 * C + hi]
        wqkv_np = np.ascontiguousarray(
            np.concatenate([wq, wk, wv], axis=1).astype(bf16))
        bq = b_attn[lo:hi]
        bk = b_attn[C + lo:C + hi]
        bv = b_attn[2 * C + lo:2 * C + hi]
        battn_np = np.ascontiguousarray(np.stack([bq, bk, bv], axis=1))  # [128, 3]
        wproj_np = np.ascontiguousarray(w_proj[lo:hi, :].astype(bf16))
        if c == 0:
            bproj_np = np.ascontiguousarray(b_proj.reshape(8, 128).T)
        else:
            bproj_np = np.zeros((128, 8), dtype=np.float32)
        in_maps.append({
            "xT": xT_np,
            "wqkv": wqkv_np,
            "battn": battn_np,
            "wproj": wproj_np,
            "bproj": bproj_np,
        })

    res = run_bass_kernel_spmd(nc, in_maps, core_ids=list(range(NCORES)),
                               trace=_trace)
    acc = res.results[0]["outT"].astype(np.float32)
    for c in range(1, NCORES):
        acc += res.results[c]["outT"].astype(np.float32)
    out = np.ascontiguousarray(acc.T).reshape(B, T, C)
    if _trace:
        kernel.last_exec_time_ns = res.exec_time_ns
        kernel.last_scope_times = res.per_core_scope_times
        kernel.last_trace = res.instructions_and_trace
    return out


# revision 10
# speedup vs baseline: 1.0300x; 1.0300x over previous
"""Causal self-attention (B=4, T=2048, C=1024, 16 heads x 64) on 8 TRN2 NeuronCores.

Sharding: tensor-parallel over heads. Core c owns heads {2c, 2c+1}:
  - w_attn column slices -> per-core QKV in transposed layout (dims on
    partitions, tokens on free dim),
  - attention in S^T form: S^T[k,q] = matmul(lhsT=kT, rhs=qT_headzeroed),
    softmax denominator via ones-columns appended to V, PV consumes exp(S^T)
    directly, partial output projection in transposed layout,
  - host sums the 8 partial projections (the TP all-reduce).

v3 (fused pipeline, engine-balanced):
  - single fused QKV+attention phase: QKV chunks of batch b+1 are
    interleaved between attention chunks of batch b, giving the PE extra
    work that covers ScalarE Exp latency (ScalarE is a hard 1 elem/cycle
    @1.2GHz with no accel modes -> ~157us of Exp; PE alone in the
    attention loop cannot outrun it, PE+QKV can),
  - ScalarE runs ONLY Exp (one merged activation per k-block: q tiles for
    both heads live in one [128,1024] psum tile),
  - causal masking via gpsimd affine_select on the exp'd probabilities
    (GpSimd is otherwise idle; DVE mask-muls removed),
  - PSUM->SBUF moves (psY->sbY) via DMA instead of DVE,
  - all PSUM evictions on DVE; deferred-by-1 normalize broadcast and
    deferred-by-2 projection keep every PE instruction's deps ready
    before the in-order PE queue reaches it,
  - bf16 operands everywhere (PSUM accum f32), f32r for the tiny
    normalize path.
"""

import sys
import numpy as np

sys.path.insert(0, "/opt/trn_rl_repo")

B, T, C = 4, 2048, 1024
NH, HD = 16, 64
NCORES = 8
TOK = B * T                 # 8192 tokens
NCH = TOK // 512            # 16 token chunks of 512
CHB = T // 512              # 4 chunks per batch
NKB_B = T // 128            # 16 k-blocks per batch
SCALE = 1.0 / 8.0

_CACHE = {}


def _build_program():
    import concourse.tile as tile
    from concourse import bacc, mybir
    from concourse.masks import make_identity

    f32 = mybir.dt.float32
    f32r = mybir.dt.float32r
    bf16 = mybir.dt.bfloat16

    nc = bacc.Bacc("TRN2", target_bir_lowering=False, debug=False,
                   num_devices=NCORES)

    xT = nc.dram_tensor("xT", [C, TOK], bf16, kind="ExternalInput").ap()
    wqkv = nc.dram_tensor("wqkv", [C, 384], bf16, kind="ExternalInput").ap()
    battn = nc.dram_tensor("battn", [128, 3], f32, kind="ExternalInput").ap()
    wproj = nc.dram_tensor("wproj", [128, C], bf16, kind="ExternalInput").ap()
    bproj = nc.dram_tensor("bproj", [128, 8], f32, kind="ExternalInput").ap()
    outT = nc.dram_tensor("outT", [C, TOK], bf16, kind="ExternalOutput").ap()

    with tile.TileContext(nc) as tc:
        with tc.tile_pool(name="const", bufs=1) as const, \
             tc.tile_pool(name="resid", bufs=1) as resid:
            w_sb = const.tile([128, 8, 384], bf16, tag="w_sb")
            nc.sync.dma_start(w_sb[:], wqkv.rearrange("(ko p) m -> p ko m", p=128))

            ident = const.tile([128, 128], f32, tag="ident")
            make_identity(nc, ident)
            bias = const.tile([128, 11], f32, tag="bias")
            battn_sb = bias[:, 0:3]
            bp_sb = bias[:, 3:11]
            nc.sync.dma_start(battn_sb, battn[:])
            nc.sync.dma_start(bp_sb, bproj[:])
            wp_sb = const.tile([128, C], bf16, tag="wp")
            nc.sync.dma_start(wp_sb[:], wproj[:])
            ones_r = const.tile([1, 128], f32r, tag="ones")
            nc.gpsimd.memset(ones_r.bitcast(f32), 1.0)

            # resident activations
            kT = resid.tile([128, NCH, 512], bf16, tag="kT")
            # qz: both heads' q in one tile; [:, :, 0:512] = head0 slots
            # (partitions 64:128 zero), [:, :, 512:1024] = head1 slots
            # (partitions 0:64 zero) -> one 1024-row S matmul per k-block
            qz = resid.tile([128, NCH, 1024], bf16, tag="qz")
            vpr = resid.tile([128, 64, 132], bf16, tag="vpr")
            vprv = vpr.rearrange("p kb (h c) -> p kb h c", c=66)

            nc.vector.memset(qz[64:128, :, 0:512], 0.0)
            nc.gpsimd.memset(qz[0:64, :, 512:1024], 0.0)
            nc.gpsimd.memset(vprv[:, :, :, 64:66], 1.0)

            xTr = xT.rearrange("(ko p) t -> p ko t", p=128)
            outTr = outT.rearrange("(od p) t -> p od t", p=128)

            with tc.tile_pool(name="xin", bufs=4) as xin, \
                 tc.tile_pool(name="vsp", bufs=2) as vsp, \
                 tc.tile_pool(name="pss", bufs=3, space="PSUM") as pssp, \
                 tc.tile_pool(name="psy", bufs=1, space="PSUM") as psyp, \
                 tc.tile_pool(name="pb", bufs=3) as pbp, \
                 tc.tile_pool(name="sby", bufs=2) as sbyp, \
                 tc.tile_pool(name="nrm", bufs=2) as nrm, \
                 tc.tile_pool(name="ytc", bufs=3) as ytc, \
                 tc.tile_pool(name="ob", bufs=2) as obp:

                def emit_A_mm(ch):
                    """QKV matmuls + q/k/v evictions for one 512-token chunk.
                    V transposes are deferred (emit_A_transp)."""
                    xa = xin.tile([128, 4, 512], bf16, tag="xc")
                    nc.sync.dma_start(xa[:], xTr[:, 0:4, ch * 512:(ch + 1) * 512])
                    xb = xin.tile([128, 4, 512], bf16, tag="xc")
                    nc.sync.dma_start(xb[:], xTr[:, 4:8, ch * 512:(ch + 1) * 512])
                    tQK = pssp.tile([128, 1024], f32, tag="s")
                    for half, m in ((0, 0), (1, 1)):
                        for ko in range(8):
                            xsrc = xa if ko < 4 else xb
                            nc.tensor.matmul(tQK[:, half * 512:(half + 1) * 512],
                                             w_sb[:, ko, m * 128:(m + 1) * 128],
                                             xsrc[:, ko % 4, :],
                                             start=(ko == 0), stop=(ko == 7))
                    nc.vector.tensor_scalar_add(qz[0:64, ch, 0:512],
                                                tQK[0:64, 0:512], battn_sb[0:64, 0:1])
                    nc.vector.tensor_scalar_add(qz[64:128, ch, 512:1024],
                                                tQK[64:128, 0:512], battn_sb[64:128, 0:1])
                    nc.vector.tensor_scalar_add(kT[:, ch, :], tQK[:, 512:1024],
                                                battn_sb[:, 1:2])
                    tV = pssp.tile([128, 1024], f32, tag="s")
                    for ko in range(8):
                        xsrc = xa if ko < 4 else xb
                        nc.tensor.matmul(tV[:, 0:512],
                                         w_sb[:, ko, 256:384],
                                         xsrc[:, ko % 4, :],
                                         start=(ko == 0), stop=(ko == 7))
                    vs = vsp.tile([128, 512], f32, tag="vs")
                    nc.vector.tensor_scalar_add(vs[:], tV[:, 0:512], battn_sb[:, 2:3])
                    return ch, vs, tV

                def emit_A_transp(ch, vs, tV):
                    # PE transposes into the back half of tV (vs eviction has
                    # long completed by the time the PE reaches these)
                    tVt = tV.rearrange("p (tb d) -> p tb d", d=128)
                    for t in range(4):
                        nc.tensor.transpose(tVt[:, 4 + t, :],
                                            vs[:, t * 128:(t + 1) * 128], ident)
                        nc.vector.tensor_copy(
                            vprv[:, ch * 4 + t, :, 0:64],
                            tVt[:, 4 + t, :].rearrange("p (h d) -> p h d", d=64))

                def emit_S(b, j, qch, kb):
                    vstart = max(0, kb * 128 - j * 512)
                    kch = b * CHB + kb // 4
                    ksub = (kb % 4) * 128
                    # matmul moving dim is capped at 512 (one PSUM bank), so
                    # one matmul per head into halves of a shared tile; the
                    # merged Exp then covers both halves in one activation
                    s = pssp.tile([128, 1024], f32, tag="s")
                    for h in range(2):
                        nc.tensor.matmul(
                            s[:, h * 512 + vstart:(h + 1) * 512],
                            kT[:, kch, ksub:ksub + 128],
                            qz[:, qch, h * 512 + vstart:(h + 1) * 512],
                            start=True, stop=True)
                    return s, vstart

                def emit_exp(j, kb, s, vstart):
                    p = pbp.tile([128, 1024], bf16, tag="p")
                    pv = p.rearrange("p (h q) -> p h q", q=512)
                    sv = s.rearrange("p (h q) -> p h q", q=512)
                    nc.scalar.activation(pv[:, :, vstart:], sv[:, :, vstart:],
                                         mybir.ActivationFunctionType.Exp, scale=SCALE)
                    if kb >= 4 * j:
                        # inline causal mask on GpSimd: keep where k <= q
                        nc.gpsimd.affine_select(
                            out=pv[:, :, vstart:vstart + 128],
                            in_=pv[:, :, vstart:vstart + 128],
                            compare_op=mybir.AluOpType.is_gt,
                            fill=0.0, base=1,
                            pattern=[[0, 2], [1, 128]], channel_multiplier=-1,
                        )
                    return p, vstart

                def emit_epilogue(qch, psY):
                    sbY = sbyp.tile([66, 1024], f32, tag="sby")
                    nc.vector.tensor_copy(sbY[:], psY[0:66, :, :])
                    sc = nrm.tile([128, 16], f32, tag="sc")
                    nc.sync.dma_start(sc[:, 0:8], sbY[64:65, :])
                    nc.vector.reciprocal(sc[:, 8:16], sc[:, 0:8])
                    rr = nrm.tile([1, 1024], f32r, tag="rr")
                    nc.sync.dma_start(rr[0:1, :], sc[:, 8:16].bitcast(f32r))
                    return sbY, rr

                def emit_norm(qch, sbY, rr):
                    r = pssp.tile([128, 1024], f32, tag="s")
                    nc.tensor.matmul(r[:, 0:512], ones_r[0:1, :], rr[0:1, 0:512],
                                     start=True, stop=True)
                    nc.tensor.matmul(r[:, 512:1024], ones_r[0:1, :], rr[0:1, 512:1024],
                                     start=True, stop=True)
                    yTch = ytc.tile([128, 512], bf16, tag="yt")
                    nc.vector.tensor_mul(yTch[0:64, :], sbY[0:64, 0:512], r[0:64, 0:512])
                    yst = nrm.tile([64, 512], bf16, tag="yst")
                    nc.vector.tensor_mul(yst[:], sbY[0:64, 512:1024], r[0:64, 512:1024])
                    nc.sync.dma_start(yTch[64:128, :], yst[:])
                    return yTch

                def emit_proj_half(yTch, oSb, half):
                    for odp in range(2):
                        tP = pssp.tile([128, 1024], f32, tag="s")
                        for h in range(2):
                            od = half * 4 + odp * 2 + h
                            nc.tensor.matmul(tP[:, h * 512:(h + 1) * 512],
                                             wp_sb[:, od * 128:(od + 1) * 128],
                                             yTch[:], start=True, stop=True)
                            nc.vector.tensor_scalar_add(oSb[:, od, :],
                                                        tP[:, h * 512:(h + 1) * 512],
                                                        bp_sb[:, od:od + 1])

                def emit_proj(qch, yTch):
                    oSb = obp.tile([128, 8, 512], bf16, tag="o")
                    emit_proj_half(yTch, oSb, 0)
                    emit_proj_half(yTch, oSb, 1)
                    nc.sync.dma_start(outTr[:, :, qch * 512:(qch + 1) * 512], oSb[:])

                # ---------------- fused schedule ----------------
                # QKV for batch 0 up front (transposes deferred by one chunk
                # so the v eviction is covered); then per attention chunk of
                # batch b, interleave one QKV chunk of batch b+1.
                pend_tr = []     # [(ch, vs, tV)] QKV chunks awaiting transposes
                pend_norm = []   # [(qch, sbY, rr)]
                pend_proj = []   # [(qch, yTch)]
                for ch in range(CHB):
                    ep = emit_A_mm(ch)
                    if pend_tr:
                        emit_A_transp(*pend_tr.pop(0))
                    pend_tr.append(ep)
                emit_A_transp(*pend_tr.pop(0))

                for b in range(B):
                    for j in range(CHB):
                        qch = b * CHB + j
                        psY = psyp.tile([128, 2, 512], f32, tag="y")
                        nkb = 4 * j + 4

                        sq = [emit_exp(j, 0, *emit_S(b, j, qch, 0)),
                              emit_exp(j, 1, *emit_S(b, j, qch, 1))]
                        for kb in range(nkb):
                            p, vstart = sq.pop(0)
                            pv = p.rearrange("p (h q) -> p h q", q=512)
                            gkb = b * NKB_B + kb
                            for h in range(2):
                                nc.tensor.matmul(psY[0:66, h, vstart:],
                                                 vprv[:, gkb, h, :],
                                                 pv[:, h, vstart:],
                                                 start=(kb == 0), stop=(kb == nkb - 1))
                            if kb + 2 < nkb:
                                sq.append(emit_exp(j, kb + 2,
                                                   *emit_S(b, j, qch, kb + 2)))

                        ep = emit_epilogue(qch, psY)
                        # interleave next batch's QKV chunk: PE work with
                        # ready deps that covers this chunk's epilogue chain
                        at = emit_A_mm(b * CHB + CHB + j) if b + 1 < B else None
                        # deferred PE work (deps ready since last iteration);
                        # ordering keeps every pssp slot's readers emitted
                        # before the slot can recycle, with the v-eviction
                        # and vpr-copy latencies covered by proj/norm matmuls
                        oproj = pend_proj.pop(0) if len(pend_proj) >= 2 else None
                        oSb = None
                        if oproj:
                            oSb = obp.tile([128, 8, 512], bf16, tag="o")
                            emit_proj_half(oproj[1], oSb, 0)
                        if at:
                            emit_A_transp(*at)
                        if pend_norm:
                            pq, psbY, prr = pend_norm.pop(0)
                            pend_proj.append((pq, emit_norm(pq, psbY, prr)))
                        if oproj:
                            emit_proj_half(oproj[1], oSb, 1)
                            nc.sync.dma_start(
                                outTr[:, :, oproj[0] * 512:(oproj[0] + 1) * 512],
                                oSb[:])
                        pend_norm.append((qch, *ep))
                # drain
                while pend_proj:
                    emit_proj(*pend_proj.pop(0))
                for pq, psbY, prr in pend_norm:
                    emit_proj(pq, emit_norm(pq, psbY, prr))

    nc.compile()
    return nc


def _get_program():
    if "nc" not in _CACHE:
        _CACHE["nc"] = _build_program()
    return _CACHE["nc"]


def kernel(x, w_attn, b_attn, w_proj, b_proj, _trace=False):
    import ml_dtypes
    from concourse.bass_utils import run_bass_kernel_spmd

    bf16 = ml_dtypes.bfloat16
    nc = _get_program()

    x = np.asarray(x, dtype=np.float32)
    w_attn = np.asarray(w_attn, dtype=np.float32)
    b_attn = np.asarray(b_attn, dtype=np.float32)
    w_proj = np.asarray(w_proj, dtype=np.float32)
    b_proj = np.asarray(b_proj, dtype=np.float32)

    xT_np = np.ascontiguousarray(x.reshape(TOK, C).T.astype(bf16))

    in_maps = []
    for c in range(NCORES):
        lo, hi = c * 128, (c + 1) * 128
        wq = w_attn[:, lo:hi]
        wk = w_attn[:, C + lo:C + hi]
        wv = w_attn[:, 2 * C + lo:2 * C + hi]
        wqkv_np = np.ascontiguousarray(
            np.concatenate([wq, wk, wv], axis=1).astype(bf16))
        bq = b_attn[lo:hi]
        bk = b_attn[C + lo:C + hi]
        bv = b_attn[2 * C + lo:2 * C + hi]
        battn_np = np.ascontiguousarray(np.stack([bq, bk, bv], axis=1))  # [128, 3]
        wproj_np = np.ascontiguousarray(w_proj[lo:hi, :].astype(bf16))
        if c == 0:
            bproj_np = np.ascontiguousarray(b_proj.reshape(8, 128).T)
        else:
            bproj_np = np.zeros((128, 8), dtype=np.float32)
        in_maps.append({
            "xT": xT_np,
            "wqkv": wqkv_np,
            "battn": battn_np,
            "wproj": wproj_np,
            "bproj": bproj_np,
        })

    res = run_bass_kernel_spmd(nc, in_maps, core_ids=list(range(NCORES)),
                               trace=_trace)
    acc = res.results[0]["outT"].astype(np.float32)
    for c in range(1, NCORES):
        acc += res.results[c]["outT"].astype(np.float32)
    out = np.ascontiguousarray(acc.T).reshape(B, T, C)
    if _trace:
        kernel.last_exec_time_ns = res.exec_time_ns
        kernel.last_scope_times = res.per_core_scope_times
        kernel.last_trace = res.instructions_and_trace
    return out
